# revision 7
# baseline (speedup 1.0000x reference)
"""Trainium2 Bass kernel for nn_NewSampler: PDF importance sampling + merge-sort.

Contract: kernel(**inputs) takes FULL inputs (rays_o, rays_d, s_vals, weights),
returns (pts, z, s) matching reference.py. Shards rays across 8 NeuronCores.

Algorithm (per ray, all on device):
  cdf = cumsum((weights[1:-1]+TINY)/sum)                    [scan]
  u_sorted (host constant, key 42)  ->  s_fine = invCDF(u)  [tagged bitonic
    merge of u with cdf + gated scans + GPSIMD local_scatter seed placement]
  z = merge(s_fine, s_vals)                                 [bitonic merge]
  pts = o + d*z                                             [ACT scale/bias]

u from jax.random.key(42) is input-independent -> host presorts it; invCDF is
monotone, so s_fine comes out sorted and the final sort is a 7-stage merge.
"""

import os
import sys
import functools

import numpy as np

for _p in ("/opt/trn_rl_repo", "/opt/pypackages"):
    if _p not in sys.path:
        sys.path.append(_p)

import concourse.bacc as bacc
import concourse.bass as bass
import concourse.tile as tile
import concourse.mybir as mybir
from concourse._compat import with_exitstack
from concourse.bass_utils import run_bass_kernel_spmd

F32 = mybir.dt.float32
I32 = mybir.dt.int32
I16 = mybir.dt.int16
OP = mybir.AluOpType
ACTF = mybir.ActivationFunctionType

TINY = 1e-6
B_FULL = 131072
NC_SAMP = 64          # coarse samples per ray
NS_OUT = 128          # output samples per ray
N_CORES = 8
BPC = B_FULL // N_CORES   # rays per core
W = 4                 # rays packed per partition
ST_RAYS = 128 * W     # rays per supertile = 512
INF = float("inf")

# ---------------------------------------------------------------- host consts


@functools.lru_cache(maxsize=1)
def _u_sorted_tagged():
    """Sorted per-ray uniforms from key 42 (input-independent), LSB set to 1
    as the merge tag (u entries tag=1, cdf entries tag=0)."""
    import jax
    import jax.numpy as jnp

    cpu = jax.devices("cpu")[0]
    with jax.default_device(cpu):
        u = jax.random.uniform(jax.random.key(42), (B_FULL, NC_SAMP),
                               dtype=jnp.float32)
        u = np.asarray(jax.device_get(u))
    us = np.sort(u, axis=-1)
    ut = (us.view(np.uint32) | np.uint32(1)).view(np.float32)
    return np.ascontiguousarray(ut)


def _build_consts():
    """(128, CF) fp32 const block, identical across partitions."""
    cols = []
    # RG248: cdf-scan reset gate, per 62-block [0, 1*61] x W
    rg248 = np.tile(np.r_[0.0, np.ones(61)], W)
    cols.append(rg248)
    # RG512: merged-scan reset gate per 128-block
    rg512 = np.tile(np.r_[0.0, np.ones(127)], W)
    cols.append(rg512)
    # POS512: q_local + 1 per 128-block
    pos512 = np.tile(np.arange(1, 129, dtype=np.float64), W)
    cols.append(pos512)
    # WB512: per 128-block constant (w*64 - 1)  [for idxB]
    wb512 = np.repeat(np.arange(W) * 64.0 - 1.0, 128)
    cols.append(wb512)
    # IDX496: per (w, k, h): 2*(w*64) + h + 1   [for idxC, +1 folds mask]
    idx496 = np.zeros(W * 62 * 2)
    for w in range(W):
        for k in range(62):
            for h in range(2):
                idx496[(w * 62 + k) * 2 + h] = 2.0 * (w * 64) + h + 1.0
    cols.append(idx496)
    cf = np.concatenate(cols).astype(np.float32)
    return np.ascontiguousarray(np.broadcast_to(cf, (128, cf.size)).copy())


_C_RG248 = 0
_C_RG512 = _C_RG248 + 248
_C_POS512 = _C_RG512 + 512
_C_WB512 = _C_POS512 + 512
_C_IDX496 = _C_WB512 + 512
_C_TOTAL = _C_IDX496 + 496


# ---------------------------------------------------------------- the program


def _block(ap, w, width, n=1):
    """cols [w*width, w*width + n*width) of a per-block-major tile view."""
    return ap[:, w * width:(w + n) * width]


@with_exitstack
def _supertile(ctx, tc, pools, dram, st, merge_split):
    """Emit one supertile (512 rays). dram: dict of DRAM APs."""
    nc = tc.nc
    io_pool, wk_pool = pools
    r0 = st * ST_RAYS

    def dslice(name, width):
        return dram[name][r0:r0 + ST_RAYS, :].rearrange(
            "(p w) c -> p (w c)", w=W)

    sv = io_pool.tile([128, W * 64], F32, tag="sv")
    wt = io_pool.tile([128, W * 64], F32, tag="wt")
    us = io_pool.tile([128, W * 64], F32, tag="us")
    od = io_pool.tile([128, W * 6], F32, tag="od")
    nc.sync.dma_start(sv[:], dslice("s_vals", 64))
    nc.sync.dma_start(wt[:], dslice("weights", 64))
    nc.sync.dma_start(us[:], dslice("u_in", 64))
    nc.sync.dma_start(od[:], dslice("od_in", 6))

    CONST = dram["_const_sb"]

    def cview(off, n):
        return CONST[:, off:off + n]

    # --- tables -----------------------------------------------------------
    # views of the W*64 layout picking 62 interior weights per block
    wt_b = wt[:].rearrange("p (w c) -> p w c", w=W)
    sv_b = sv[:].rearrange("p (w c) -> p w c", w=W)

    wp = wk_pool.tile([128, W * 62], F32, tag="wp")
    wp_b = wp[:].rearrange("p (w c) -> p w c", w=W)
    nc.vector.tensor_scalar(wp_b, wt_b[:, :, 1:63], TINY, None, OP.add)

    tsum = wk_pool.tile([128, W], F32, tag="tsum")
    nc.vector.tensor_reduce(tsum[:].rearrange("p (w c) -> p w c", c=1),
                            wp_b, mybir.AxisListType.X, OP.add)
    rts = wk_pool.tile([128, W], F32, tag="rts")
    nc.vector.reciprocal(rts[:], tsum[:])

    pdf = wk_pool.tile([128, W * 62], F32, tag="pdf")
    # broadcast rT along the 62-dim: iterate (k outer, w inner) so that the
    # broadcast operand keeps innermost stride 1
    pdf_kw = pdf[:].rearrange("p (w c) -> p c w", w=W)
    wp_kw = wp[:].rearrange("p (w c) -> p c w", w=W)
    rts_kw = rts[:].unsqueeze(1).broadcast_to([128, 62, W])
    nc.vector.tensor_tensor(pdf_kw, wp_kw, rts_kw, OP.mult)

    cdf = wk_pool.tile([128, W * 62], F32, tag="cdf")
    nc.vector.tensor_tensor_scan(cdf[:], cview(_C_RG248, 248), pdf[:], 0.0,
                                 OP.mult, OP.add)
    cdf_b = cdf[:].rearrange("p (w c) -> p w c", w=W)

    # mid63: 0.5*(sv[k]+sv[k+1]), k=0..62 per block
    svh = wk_pool.tile([128, W * 64], F32, tag="svh")
    nc.scalar.activation(svh[:], sv[:], ACTF.Copy, scale=0.5)
    svh_b = svh[:].rearrange("p (w c) -> p w c", w=W)
    mid63 = wk_pool.tile([128, W * 63], F32, tag="mid63")
    mid63_b = mid63[:].rearrange("p (w c) -> p w c", w=W)
    nc.vector.tensor_tensor(mid63_b, svh_b[:, :, :63], svh_b[:, :, 1:],
                            OP.add)

    # mid-seed (contiguous k=0..61) for scatter
    midseed = wk_pool.tile([128, W * 62], F32, tag="midseed")
    midseed_b = midseed[:].rearrange("p (w c) -> p w c", w=W)
    nc.scalar.copy(midseed_b, mid63_b[:, :, :62])

    # dmid_k = mid[k+1] - mid[k] + TINY
    dmid = wk_pool.tile([128, W * 62], F32, tag="dmid")
    dmid_b = dmid[:].rearrange("p (w c) -> p w c", w=W)
    nc.vector.scalar_tensor_tensor(dmid_b, mid63_b[:, :, 1:], TINY,
                                   mid63_b[:, :, :62], OP.add, OP.subtract)

    # cklo = [0, cdf_1..cdf_61] per block (contiguous, doubles as C-seed)
    cklo = wk_pool.tile([128, W * 62], F32, tag="cklo")
    cklo_b = cklo[:].rearrange("p (w c) -> p w c", w=W)
    nc.vector.memset(cklo_b[:, :, 0:1], 0.0)
    nc.scalar.copy(cklo_b[:, :, 1:], cdf_b[:, :, :61])

    # D_k = cdf_{k+1} - C_k ; Deff = denom<TINY ? 1 : denom
    dd = wk_pool.tile([128, W * 62], F32, tag="dd")
    nc.gpsimd.tensor_tensor(dd[:].rearrange("p (w c) -> p w c", w=W),
                            cdf_b, cklo_b, OP.subtract)
    deg = wk_pool.tile([128, W * 62], F32, tag="deg")
    nc.gpsimd.tensor_scalar(deg[:], dd[:], TINY, None, OP.is_lt)
    onemd = wk_pool.tile([128, W * 62], F32, tag="onemd")
    nc.gpsimd.tensor_scalar(onemd[:], dd[:], -1.0, 1.0, OP.mult, OP.add)
    degd = wk_pool.tile([128, W * 62], F32, tag="degd")
    nc.gpsimd.tensor_tensor(degd[:], deg[:], onemd[:], OP.mult)
    deff = wk_pool.tile([128, W * 62], F32, tag="deff")
    nc.gpsimd.tensor_tensor(deff[:], dd[:], degd[:], OP.add)

    rdd = wk_pool.tile([128, W * 62], F32, tag="rdd")
    rscr = wk_pool.tile([128, W * 62], F32, tag="rscr")
    nc.vector.reciprocal_approx_accurate(rdd[:], deff[:], rscr[:])

    aseed = wk_pool.tile([128, W * 62], F32, tag="aseed")
    nc.vector.tensor_tensor(aseed[:], dmid[:], rdd[:], OP.mult)

    # --- merge #1: u (tag LSB=1) vs cdf_1..61 (tag LSB=0) -----------------
    m1a = wk_pool.tile([128, W * 128], F32, tag="m1a")
    m1b = wk_pool.tile([128, W * 128], F32, tag="m1b")
    m1a_b = m1a[:].rearrange("p (w c) -> p w c", w=W)
    # Q half: u already tagged host-side
    nc.scalar.copy(m1a_b[:, :, 0:64], us[:].rearrange("p (w c) -> p w c", w=W))
    # C half: [inf, inf, inf, cdf_61..cdf_1] ; clear LSB as tag 0
    nc.vector.memset(m1a_b[:, :, 64:67], INF)
    crev = cdf_b[:, :, 60::-1]  # cdf_61 .. cdf_1
    nc.vector.tensor_scalar(
        m1a_b.bitcast(I32)[:, :, 67:], crev.bitcast(I32), -2, None,
        OP.bitwise_and)

    bufs = [m1a, m1b]
    cur = 0
    for si, d in enumerate([64, 32, 16, 8, 4, 2, 1]):
        src = bufs[cur][:].rearrange("p (w b two d) -> p w b two d",
                                     w=W, two=2, d=d)
        dst = bufs[1 - cur][:].rearrange("p (w b two d) -> p w b two d",
                                         w=W, two=2, d=d)
        lo, hi = src[:, :, :, 0, :], src[:, :, :, 1, :]
        eng_mn, eng_mx = merge_split[si]
        eng_mn.tensor_tensor(dst[:, :, :, 0, :], lo, hi, OP.min)
        eng_mx.tensor_tensor(dst[:, :, :, 1, :], lo, hi, OP.max)
        cur = 1 - cur
    mkeys = bufs[cur]  # merged (128, W*128)

    # --- post-merge scans --------------------------------------------------
    tagi = wk_pool.tile([128, W * 128], I32, tag="tagi")
    nc.vector.tensor_scalar(tagi[:], mkeys[:].bitcast(I32), 1, None,
                            OP.bitwise_and)
    tagq = wk_pool.tile([128, W * 128], F32, tag="tagq")   # 1 at u slots
    nc.scalar.copy(tagq[:], tagi[:])
    tagc = wk_pool.tile([128, W * 128], F32, tag="tagc")   # 1 at cdf slots
    nc.vector.tensor_scalar(tagc[:], tagq[:], -1.0, 1.0, OP.mult, OP.add)

    kcount = wk_pool.tile([128, W * 128], F32, tag="kcount")
    nc.vector.tensor_tensor_scan(kcount[:], cview(_C_RG512, 512), tagc[:],
                                 0.0, OP.mult, OP.add)
    icount = wk_pool.tile([128, W * 128], F32, tag="icount")
    nc.gpsimd.tensor_tensor(icount[:], cview(_C_POS512, 512), kcount[:],
                            OP.subtract)

    # idxB = tagc ? (w*64 + kcount - 1) : -1
    t1 = wk_pool.tile([128, W * 128], F32, tag="t1")
    nc.gpsimd.tensor_tensor(t1[:], kcount[:], cview(_C_WB512, 512), OP.add)
    t2 = wk_pool.tile([128, W * 128], F32, tag="t2")
    nc.gpsimd.tensor_tensor(t2[:], tagc[:], t1[:], OP.mult)
    idxbf = wk_pool.tile([128, W * 128], F32, tag="idxbf")
    nc.vector.scalar_tensor_tensor(idxbf[:], tagc[:], -1.0, t2[:],
                                   OP.add, OP.add)
    idxb16 = wk_pool.tile([128, W * 128], I16, tag="idxb16")
    nc.scalar.copy(idxb16[:], idxbf[:])
    datb16 = wk_pool.tile([128, W * 128], I16, tag="datb16")
    nc.scalar.copy(datb16[:], icount[:])

    f16 = wk_pool.tile([128, W * 64], I16, tag="f16")
    nc.gpsimd.local_scatter(f16[:], datb16[:], idxb16[:], channels=128,
                            num_elems=W * 64, num_idxs=W * 128)

    # posf: per block 63 cols: [0, F_1..F_61, 64.0]
    posf = wk_pool.tile([128, W * 63], F32, tag="posf")
    posf_b = posf[:].rearrange("p (w c) -> p w c", w=W)
    nc.vector.memset(posf_b[:, :, 0:1], 0.0)
    nc.vector.memset(posf_b[:, :, 62:63], 64.0)
    f16_b = f16[:].rearrange("p (w c) -> p w c", w=W)
    nc.scalar.copy(posf_b[:, :, 1:62], f16_b[:, :, 0:61])

    ne = wk_pool.tile([128, W * 62], F32, tag="ne")
    nc.vector.tensor_tensor(ne[:].rearrange("p (w c) -> p w c", w=W),
                            posf_b[:, :, :62], posf_b[:, :, 1:], OP.is_lt)

    # idxC halves: ne ? 2*(w*64 + pos_k) + h : -1   (shared by 3 scatters)
    pos2 = wk_pool.tile([128, W * 62], F32, tag="pos2")
    nc.gpsimd.tensor_scalar(pos2[:].rearrange("p (w c) -> p w c", w=W),
                            posf_b[:, :, :62], 2.0, None, OP.mult)
    idxcf = wk_pool.tile([128, W * 124], F32, tag="idxcf")
    idxcf_h = idxcf[:].rearrange("p (w c h) -> p (w c) h", h=2, w=W)
    pos2_h = pos2[:].unsqueeze(2).broadcast_to([128, W * 62, 2])
    nc.vector.tensor_tensor(idxcf_h, pos2_h,
                            cview(_C_IDX496, 496).rearrange(
                                "p (c h) -> p c h", h=2), OP.add)
    ne_h = ne[:].unsqueeze(2).broadcast_to([128, W * 62, 2])
    idxcm = wk_pool.tile([128, W * 124], F32, tag="idxcm")
    nc.gpsimd.tensor_tensor(idxcm[:].rearrange("p (c h) -> p c h", h=2),
                            idxcf_h, ne_h, OP.mult)
    idxcf2 = wk_pool.tile([128, W * 124], F32, tag="idxcf2")
    nc.vector.tensor_scalar(idxcf2[:], idxcm[:], -1.0, None, OP.add)
    idxc16 = wk_pool.tile([128, W * 124], I16, tag="idxc16")
    nc.scalar.copy(idxc16[:], idxcf2[:])

    # three seed scatters into i-space (64 fp32 = 128 halves per block)
    sca = wk_pool.tile([128, W * 64], F32, tag="sca")
    scm = wk_pool.tile([128, W * 64], F32, tag="scm")
    scc = wk_pool.tile([128, W * 64], F32, tag="scc")
    for dst_t, src_t in ((sca, aseed), (scm, midseed), (scc, cklo)):
        nc.gpsimd.local_scatter(dst_t[:].bitcast(I16), src_t[:].bitcast(I16),
                                idxc16[:], channels=128, num_elems=W * 128,
                                num_idxs=W * 124)

    # gate / propagate
    gate = wk_pool.tile([128, W * 64], F32, tag="gate")
    nc.gpsimd.tensor_scalar(gate[:], scm[:], 0.0, None, OP.is_gt)
    ng = wk_pool.tile([128, W * 64], F32, tag="ng")
    nc.vector.tensor_scalar(ng[:], gate[:], -1.0, 1.0, OP.mult, OP.add)
    ap_ = wk_pool.tile([128, W * 64], F32, tag="ap_")
    mp_ = wk_pool.tile([128, W * 64], F32, tag="mp_")
    cp_ = wk_pool.tile([128, W * 64], F32, tag="cp_")
    nc.vector.tensor_tensor_scan(ap_[:], ng[:], sca[:], 0.0, OP.mult, OP.add)
    nc.vector.tensor_tensor_scan(mp_[:], ng[:], scm[:], 0.0, OP.mult, OP.add)
    nc.vector.tensor_tensor_scan(cp_[:], ng[:], scc[:], 0.0, OP.mult, OP.add)

    # x = mp + (u - cp) * ap   -> write into merge2 buffer Q half
    xt = wk_pool.tile([128, W * 64], F32, tag="xt")
    nc.gpsimd.tensor_tensor(xt[:], us[:], cp_[:], OP.subtract)
    xta = wk_pool.tile([128, W * 64], F32, tag="xta")
    nc.gpsimd.tensor_tensor(xta[:], xt[:], ap_[:], OP.mult)
    m2a = wk_pool.tile([128, W * 128], F32, tag="m2a")
    m2b = wk_pool.tile([128, W * 128], F32, tag="m2b")
    m2a_b = m2a[:].rearrange("p (w c) -> p w c", w=W)
    nc.vector.tensor_tensor(m2a_b[:, :, :64],
                            xta[:].rearrange("p (w c) -> p w c", w=W),
                            mp_[:].rearrange("p (w c) -> p w c", w=W), OP.add)
    # upper half: s_vals reversed (descending)
    nc.scalar.copy(m2a_b[:, :, 64:], sv_b[:, :, ::-1])

    bufs2 = [m2a, m2b]
    cur = 0
    for si, d in enumerate([64, 32, 16, 8, 4, 2, 1]):
        src = bufs2[cur][:].rearrange("p (w b two d) -> p w b two d",
                                      w=W, two=2, d=d)
        dst = bufs2[1 - cur][:].rearrange("p (w b two d) -> p w b two d",
                                          w=W, two=2, d=d)
        lo, hi = src[:, :, :, 0, :], src[:, :, :, 1, :]
        eng_mn, eng_mx = merge_split[si]
        eng_mn.tensor_tensor(dst[:, :, :, 0, :], lo, hi, OP.min)
        eng_mx.tensor_tensor(dst[:, :, :, 1, :], lo, hi, OP.max)
        cur = 1 - cur
    zf = bufs2[cur]

    # --- outputs ----------------------------------------------------------
    zslice = dram["z_out"][r0:r0 + ST_RAYS, :].rearrange(
        "(p w) c -> p (w c)", w=W)
    sslice = dram["s_out"][r0:r0 + ST_RAYS, :].rearrange(
        "(p w) c -> p (w c)", w=W)
    nc.sync.dma_start(zslice, zf[:])
    nc.sync.dma_start(sslice, zf[:])

    pts = wk_pool.tile([128, W * 384], F32, tag="pts")
    zf_b = zf[:].rearrange("p (w c) -> p w c", w=W)
    pts_w = pts[:].rearrange("p (w k c) -> p w k c", w=W, c=3)
    for w in range(W):
        for c in range(3):
            nc.scalar.activation(pts_w[:, w, :, c], zf_b[:, w, :],
                                 ACTF.Identity,
                                 bias=od[:, w * 6 + c:w * 6 + c + 1],
                                 scale=od[:, w * 6 + 3 + c:w * 6 + 4 + c])
    ptsl = dram["pts_out"][r0:r0 + ST_RAYS, :].rearrange(
        "(p w) c -> p (w c)", w=W)
    nc.sync.dma_start(ptsl, pts[:])


def build_program(n_rays):
    """Build + compile the per-core program for n_rays rays."""
    assert n_rays % ST_RAYS == 0
    nst = n_rays // ST_RAYS
    nc = bacc.Bacc("TRN2", target_bir_lowering=False, debug=False,
                   enable_asserts=False)
    dram = {}
    for name, width in (("s_vals", 64), ("weights", 64), ("u_in", 64),
                        ("od_in", 6)):
        dram[name] = nc.dram_tensor(name, [n_rays, width], F32,
                                    kind="ExternalInput").ap()
    dram["_const"] = nc.dram_tensor("consts", [128, _C_TOTAL], F32,
                                    kind="ExternalInput").ap()
    for name, width in (("z_out", 128), ("s_out", 128), ("pts_out", 384)):
        dram[name] = nc.dram_tensor(name, [n_rays, width], F32,
                                    kind="ExternalOutput").ap()

    with tile.TileContext(nc) as tc:
        with (tc.tile_pool(name="io", bufs=4) as io_pool,
              tc.tile_pool(name="wk", bufs=2) as wk_pool,
              tc.tile_pool(name="cn", bufs=1) as cn_pool):
            cb = cn_pool.tile([128, _C_TOTAL], F32, tag="cb")
            nc.sync.dma_start(cb[:], dram["_const"])
            dram["_const_sb"] = cb[:]
            # merge stage engine split: (min_engine, max_engine) per stage
            v = nc.vector
            merge_split = [(v, v)] * 7
            for st in range(nst):
                _supertile(tc, pools=(io_pool, wk_pool), dram=dram, st=st,
                           merge_split=merge_split)
    nc.compile()
    return nc


@functools.lru_cache(maxsize=2)
def _compiled(n_rays):
    return build_program(n_rays)


# ---------------------------------------------------------------- entry point


def _run(inputs, n_cores=N_CORES, trace=False, trace_kwargs=None):
    rays_o = np.ascontiguousarray(inputs["rays_o"], dtype=np.float32)
    rays_d = np.ascontiguousarray(inputs["rays_d"], dtype=np.float32)
    s_vals = np.ascontiguousarray(inputs["s_vals"], dtype=np.float32)
    weights = np.ascontiguousarray(inputs["weights"], dtype=np.float32)
    b = s_vals.shape[0]
    bpc = b // n_cores
    od = np.concatenate([rays_o, rays_d], axis=1)
    u_t = _u_sorted_tagged()[:b]
    consts = _build_consts()

    nc = _compiled(bpc)
    in_maps = []
    for c in range(n_cores):
        sl = slice(c * bpc, (c + 1) * bpc)
        in_maps.append({
            "s_vals": s_vals[sl],
            "weights": weights[sl],
            "u_in": u_t[sl],
            "od_in": od[sl],
            "consts": consts,
        })
    res = run_bass_kernel_spmd(nc, in_maps, list(range(n_cores)),
                               trace=trace, **(trace_kwargs or {}))
    z = np.concatenate([r["z_out"] for r in res.results], axis=0)
    s = np.concatenate([r["s_out"] for r in res.results], axis=0)
    pts = np.concatenate([r["pts_out"] for r in res.results],
                         axis=0).reshape(b, NS_OUT, 3)
    return (pts, z, s), res


def kernel(rays_o, rays_d, s_vals, weights):
    (pts, z, s), _ = _run({"rays_o": rays_o, "rays_d": rays_d,
                           "s_vals": s_vals, "weights": weights})
    return pts, z, s


# revision 9
# speedup vs baseline: 1.6766x; 1.6766x over previous
"""Trainium2 Bass kernel for nn_NewSampler: PDF importance sampling + merge-sort.

Contract: kernel(**inputs) takes FULL inputs (rays_o, rays_d, s_vals, weights),
returns (pts, z, s) matching reference.py. Shards rays across 8 NeuronCores.

Algorithm (per ray, all on device):
  cdf = cumsum((weights[1:-1]+TINY)/sum)                    [scan]
  u_sorted (host constant, key 42)  ->  s_fine = invCDF(u)  [tagged bitonic
    merge of u with cdf + gated scans + GPSIMD local_scatter seed placement]
  z = merge(s_fine, s_vals)                                 [bitonic merge]
  pts = o + d*z                                             [ACT scale/bias]

u from jax.random.key(42) is input-independent -> host presorts it; invCDF is
monotone, so s_fine comes out sorted and the final sort is a 7-stage merge.
"""

import os
import sys
import functools

import numpy as np

for _p in ("/opt/trn_rl_repo", "/opt/pypackages"):
    if _p not in sys.path:
        sys.path.append(_p)

import concourse.bacc as bacc
import concourse.bass as bass
import concourse.tile as tile
import concourse.mybir as mybir
from concourse._compat import with_exitstack
from concourse.bass_utils import run_bass_kernel_spmd

F32 = mybir.dt.float32
I32 = mybir.dt.int32
I16 = mybir.dt.int16
OP = mybir.AluOpType
ACTF = mybir.ActivationFunctionType

TINY = 1e-6
B_FULL = 131072
NC_SAMP = 64          # coarse samples per ray
NS_OUT = 128          # output samples per ray
N_CORES = 8
BPC = B_FULL // N_CORES   # rays per core
W = 4                 # rays packed per partition
ST_RAYS = 128 * W     # rays per supertile = 512
INF = float("inf")

# ---------------------------------------------------------------- host consts


@functools.lru_cache(maxsize=1)
def _u_sorted_tagged():
    """Sorted per-ray uniforms from key 42 (input-independent), LSB set to 1
    as the merge tag (u entries tag=1, cdf entries tag=0)."""
    import jax
    import jax.numpy as jnp

    cpu = jax.devices("cpu")[0]
    with jax.default_device(cpu):
        u = jax.random.uniform(jax.random.key(42), (B_FULL, NC_SAMP),
                               dtype=jnp.float32)
        u = np.asarray(jax.device_get(u))
    us = np.sort(u, axis=-1)
    ut = (us.view(np.uint32) | np.uint32(1)).view(np.float32)
    return np.ascontiguousarray(ut)


def _build_consts():
    """(128, CF) fp32 const block, identical across partitions."""
    cols = []
    # RG248: cdf-scan reset gate, per 62-block [0, 1*61] x W
    rg248 = np.tile(np.r_[0.0, np.ones(61)], W)
    cols.append(rg248)
    # RG512: merged-scan reset gate per 128-block
    rg512 = np.tile(np.r_[0.0, np.ones(127)], W)
    cols.append(rg512)
    # POS512: q_local + 1 per 128-block
    pos512 = np.tile(np.arange(1, 129, dtype=np.float64), W)
    cols.append(pos512)
    # WB512: per 128-block constant (w*64 - 1)  [for idxB]
    wb512 = np.repeat(np.arange(W) * 64.0 - 1.0, 128)
    cols.append(wb512)
    # IDX496: per (w, k, h): 2*(w*64) + h + 1   [for idxC, +1 folds mask]
    idx496 = np.zeros(W * 62 * 2)
    for w in range(W):
        for k in range(62):
            for h in range(2):
                idx496[(w * 62 + k) * 2 + h] = 2.0 * (w * 64) + h + 1.0
    cols.append(idx496)
    cf = np.concatenate(cols).astype(np.float32)
    return np.ascontiguousarray(np.broadcast_to(cf, (128, cf.size)).copy())


_C_RG248 = 0
_C_RG512 = _C_RG248 + 248
_C_POS512 = _C_RG512 + 512
_C_WB512 = _C_POS512 + 512
_C_IDX496 = _C_WB512 + 512
_C_TOTAL = _C_IDX496 + 496


# ---------------------------------------------------------------- the program


def _block(ap, w, width, n=1):
    """cols [w*width, w*width + n*width) of a per-block-major tile view."""
    return ap[:, w * width:(w + n) * width]


@with_exitstack
def _supertile(ctx, tc, pools, dram, st, merge_split):
    """Emit one supertile (512 rays). dram: dict of DRAM APs."""
    nc = tc.nc
    io_pool, wk_pool = pools
    r0 = st * ST_RAYS

    def dslice(name, width):
        return dram[name][r0:r0 + ST_RAYS, :].rearrange(
            "(p w) c -> p (w c)", w=W)

    sv = io_pool.tile([128, W * 64], F32, tag="sv")
    wt = io_pool.tile([128, W * 64], F32, tag="wt")
    us = io_pool.tile([128, W * 64], F32, tag="us")
    od = io_pool.tile([128, W * 6], F32, tag="od")
    nc.sync.dma_start(sv[:], dslice("s_vals", 64))
    nc.sync.dma_start(wt[:], dslice("weights", 64))
    nc.sync.dma_start(us[:], dslice("u_in", 64))
    nc.sync.dma_start(od[:], dslice("od_in", 6))

    CONST = dram["_const_sb"]

    def cview(off, n):
        return CONST[:, off:off + n]

    # --- tables -----------------------------------------------------------
    # views of the W*64 layout picking 62 interior weights per block
    wt_b = wt[:].rearrange("p (w c) -> p w c", w=W)
    sv_b = sv[:].rearrange("p (w c) -> p w c", w=W)

    wp = wk_pool.tile([128, W * 62], F32, tag="wp")
    wp_b = wp[:].rearrange("p (w c) -> p w c", w=W)
    nc.vector.tensor_scalar(wp_b, wt_b[:, :, 1:63], TINY, None, OP.add)

    tsum = wk_pool.tile([128, W], F32, tag="tsum")
    nc.vector.tensor_reduce(tsum[:].rearrange("p (w c) -> p w c", c=1),
                            wp_b, mybir.AxisListType.X, OP.add)
    rts = wk_pool.tile([128, W], F32, tag="rts")
    nc.vector.reciprocal(rts[:], tsum[:])

    pdf = wk_pool.tile([128, W * 62], F32, tag="pdf")
    # broadcast rT along the 62-dim: iterate (k outer, w inner) so that the
    # broadcast operand keeps innermost stride 1
    pdf_kw = pdf[:].rearrange("p (w c) -> p c w", w=W)
    wp_kw = wp[:].rearrange("p (w c) -> p c w", w=W)
    rts_kw = rts[:].unsqueeze(1).broadcast_to([128, 62, W])
    nc.vector.tensor_tensor(pdf_kw, wp_kw, rts_kw, OP.mult)

    cdf = wk_pool.tile([128, W * 62], F32, tag="cdf")
    nc.vector.tensor_tensor_scan(cdf[:], cview(_C_RG248, 248), pdf[:], 0.0,
                                 OP.mult, OP.add)
    cdf_b = cdf[:].rearrange("p (w c) -> p w c", w=W)

    # mid63: 0.5*(sv[k]+sv[k+1]), k=0..62 per block
    svh = wk_pool.tile([128, W * 64], F32, tag="svh")
    nc.scalar.activation(svh[:], sv[:], ACTF.Copy, scale=0.5)
    svh_b = svh[:].rearrange("p (w c) -> p w c", w=W)
    mid63 = wk_pool.tile([128, W * 63], F32, tag="mid63")
    mid63_b = mid63[:].rearrange("p (w c) -> p w c", w=W)
    nc.vector.tensor_tensor(mid63_b, svh_b[:, :, :63], svh_b[:, :, 1:],
                            OP.add)

    # mid-seed (contiguous k=0..61) for scatter
    midseed = wk_pool.tile([128, W * 62], F32, tag="midseed")
    midseed_b = midseed[:].rearrange("p (w c) -> p w c", w=W)
    nc.scalar.copy(midseed_b, mid63_b[:, :, :62])

    # dmid_k = mid[k+1] - mid[k] + TINY
    dmid = wk_pool.tile([128, W * 62], F32, tag="dmid")
    dmid_b = dmid[:].rearrange("p (w c) -> p w c", w=W)
    nc.vector.scalar_tensor_tensor(dmid_b, mid63_b[:, :, 1:], TINY,
                                   mid63_b[:, :, :62], OP.add, OP.subtract)

    # cklo = [0, cdf_1..cdf_61] per block (contiguous, doubles as C-seed)
    cklo = wk_pool.tile([128, W * 62], F32, tag="cklo")
    cklo_b = cklo[:].rearrange("p (w c) -> p w c", w=W)
    nc.vector.memset(cklo_b[:, :, 0:1], 0.0)
    nc.scalar.copy(cklo_b[:, :, 1:], cdf_b[:, :, :61])

    # D_k = cdf_{k+1} - C_k ; Deff = denom<TINY ? 1 : denom
    dd = wk_pool.tile([128, W * 62], F32, tag="dd")
    nc.vector.tensor_tensor(dd[:].rearrange("p (w c) -> p w c", w=W),
                            cdf_b, cklo_b, OP.subtract)
    deg = wk_pool.tile([128, W * 62], F32, tag="deg")
    nc.vector.tensor_scalar(deg[:], dd[:], TINY, None, OP.is_lt)
    onemd = wk_pool.tile([128, W * 62], F32, tag="onemd")
    nc.vector.tensor_scalar(onemd[:], dd[:], -1.0, 1.0, OP.mult, OP.add)
    degd = wk_pool.tile([128, W * 62], F32, tag="degd")
    nc.vector.tensor_tensor(degd[:], deg[:], onemd[:], OP.mult)
    deff = wk_pool.tile([128, W * 62], F32, tag="deff")
    nc.vector.tensor_tensor(deff[:], dd[:], degd[:], OP.add)

    rdd = wk_pool.tile([128, W * 62], F32, tag="rdd")
    rscr = wk_pool.tile([128, W * 62], F32, tag="rscr")
    nc.vector.reciprocal_approx_accurate(rdd[:], deff[:], rscr[:])

    aseed = wk_pool.tile([128, W * 62], F32, tag="aseed")
    nc.vector.tensor_tensor(aseed[:], dmid[:], rdd[:], OP.mult)

    # --- merge #1: u (tag LSB=1) vs cdf_1..61 (tag LSB=0) -----------------
    m1a = wk_pool.tile([128, W * 128], F32, tag="m1a")
    m1b = wk_pool.tile([128, W * 128], F32, tag="m1b")
    m1a_b = m1a[:].rearrange("p (w c) -> p w c", w=W)
    # Q half: u already tagged host-side
    nc.scalar.copy(m1a_b[:, :, 0:64], us[:].rearrange("p (w c) -> p w c", w=W))
    # C half: [inf, inf, inf, cdf_61..cdf_1] ; clear LSB as tag 0
    nc.vector.memset(m1a_b[:, :, 64:67], INF)
    crev = cdf_b[:, :, 60::-1]  # cdf_61 .. cdf_1
    nc.vector.tensor_scalar(
        m1a_b.bitcast(I32)[:, :, 67:], crev.bitcast(I32), -2, None,
        OP.bitwise_and)

    bufs = [m1a, m1b]
    cur = 0
    for si, d in enumerate([64, 32, 16, 8, 4, 2, 1]):
        src = bufs[cur][:].rearrange("p (w b two d) -> p w b two d",
                                     w=W, two=2, d=d)
        dst = bufs[1 - cur][:].rearrange("p (w b two d) -> p w b two d",
                                         w=W, two=2, d=d)
        lo, hi = src[:, :, :, 0, :], src[:, :, :, 1, :]
        eng_mn, eng_mx = merge_split[si]
        eng_mn.tensor_tensor(dst[:, :, :, 0, :], lo, hi, OP.min)
        eng_mx.tensor_tensor(dst[:, :, :, 1, :], lo, hi, OP.max)
        cur = 1 - cur
    mkeys = bufs[cur]  # merged (128, W*128)

    # --- post-merge scans --------------------------------------------------
    tagi = wk_pool.tile([128, W * 128], I32, tag="tagi")
    nc.vector.tensor_scalar(tagi[:], mkeys[:].bitcast(I32), 1, 1,
                            OP.bitwise_and, OP.bitwise_xor)
    tagc = wk_pool.tile([128, W * 128], F32, tag="tagc")   # 1 at cdf slots
    nc.scalar.copy(tagc[:], tagi[:])

    kcount = wk_pool.tile([128, W * 128], F32, tag="kcount")
    nc.vector.tensor_tensor_scan(kcount[:], cview(_C_RG512, 512), tagc[:],
                                 0.0, OP.mult, OP.add)
    icount = wk_pool.tile([128, W * 128], F32, tag="icount")
    nc.vector.tensor_tensor(icount[:], cview(_C_POS512, 512), kcount[:],
                            OP.subtract)

    # idxB = tagc ? (w*64 + kcount - 1) : -1
    t1 = wk_pool.tile([128, W * 128], F32, tag="t1")
    nc.vector.tensor_tensor(t1[:], kcount[:], cview(_C_WB512, 512), OP.add)
    t2 = wk_pool.tile([128, W * 128], F32, tag="t2")
    nc.vector.tensor_tensor(t2[:], tagc[:], t1[:], OP.mult)
    idxbf = wk_pool.tile([128, W * 128], F32, tag="idxbf")
    nc.vector.scalar_tensor_tensor(idxbf[:], tagc[:], -1.0, t2[:],
                                   OP.add, OP.add)
    idxb16 = wk_pool.tile([128, W * 128], I16, tag="idxb16")
    nc.scalar.copy(idxb16[:], idxbf[:])
    datb16 = wk_pool.tile([128, W * 128], I16, tag="datb16")
    nc.scalar.copy(datb16[:], icount[:])

    f16 = wk_pool.tile([128, W * 64], I16, tag="f16")
    nc.gpsimd.local_scatter(f16[:], datb16[:], idxb16[:], channels=128,
                            num_elems=W * 64, num_idxs=W * 128)

    # posf: per block 63 cols: [0, F_1..F_61, 64.0]
    posf = wk_pool.tile([128, W * 63], F32, tag="posf")
    posf_b = posf[:].rearrange("p (w c) -> p w c", w=W)
    nc.vector.memset(posf_b[:, :, 0:1], 0.0)
    nc.vector.memset(posf_b[:, :, 62:63], 64.0)
    f16_b = f16[:].rearrange("p (w c) -> p w c", w=W)
    nc.scalar.copy(posf_b[:, :, 1:62], f16_b[:, :, 0:61])

    ne = wk_pool.tile([128, W * 62], F32, tag="ne")
    nc.vector.tensor_tensor(ne[:].rearrange("p (w c) -> p w c", w=W),
                            posf_b[:, :, :62], posf_b[:, :, 1:], OP.is_lt)

    # idxC halves: ne ? 2*(w*64 + pos_k) + h : -1   (shared by 3 scatters)
    pos2 = wk_pool.tile([128, W * 62], F32, tag="pos2")
    nc.vector.tensor_scalar(pos2[:].rearrange("p (w c) -> p w c", w=W),
                            posf_b[:, :, :62], 2.0, None, OP.mult)
    idxcf = wk_pool.tile([128, W * 124], F32, tag="idxcf")
    idxcf_h = idxcf[:].rearrange("p (w c h) -> p (w c) h", h=2, w=W)
    pos2_h = pos2[:].unsqueeze(2).broadcast_to([128, W * 62, 2])
    nc.vector.tensor_tensor(idxcf_h, pos2_h,
                            cview(_C_IDX496, 496).rearrange(
                                "p (c h) -> p c h", h=2), OP.add)
    ne_h = ne[:].unsqueeze(2).broadcast_to([128, W * 62, 2])
    idxcm = wk_pool.tile([128, W * 124], F32, tag="idxcm")
    nc.vector.tensor_tensor(idxcm[:].rearrange("p (c h) -> p c h", h=2),
                            idxcf_h, ne_h, OP.mult)
    idxc16 = wk_pool.tile([128, W * 124], I16, tag="idxc16")
    nc.scalar.activation(idxc16[:], idxcm[:], ACTF.Identity,
                         bias=dram["_neg1"])

    # three seed scatters into i-space (64 fp32 = 128 halves per block)
    sca = wk_pool.tile([128, W * 64], F32, tag="sca")
    scm = wk_pool.tile([128, W * 64], F32, tag="scm")
    scc = wk_pool.tile([128, W * 64], F32, tag="scc")
    for dst_t, src_t in ((sca, aseed), (scm, midseed), (scc, cklo)):
        nc.gpsimd.local_scatter(dst_t[:].bitcast(I16), src_t[:].bitcast(I16),
                                idxc16[:], channels=128, num_elems=W * 128,
                                num_idxs=W * 124)

    # gate / propagate
    gate = wk_pool.tile([128, W * 64], F32, tag="gate")
    nc.vector.tensor_scalar(gate[:], scm[:], 0.0, None, OP.is_gt)
    ng = wk_pool.tile([128, W * 64], F32, tag="ng")
    nc.vector.tensor_scalar(ng[:], gate[:], -1.0, 1.0, OP.mult, OP.add)
    ap_ = wk_pool.tile([128, W * 64], F32, tag="ap_")
    mp_ = wk_pool.tile([128, W * 64], F32, tag="mp_")
    cp_ = wk_pool.tile([128, W * 64], F32, tag="cp_")
    nc.vector.tensor_tensor_scan(ap_[:], ng[:], sca[:], 0.0, OP.mult, OP.add)
    nc.vector.tensor_tensor_scan(mp_[:], ng[:], scm[:], 0.0, OP.mult, OP.add)
    nc.vector.tensor_tensor_scan(cp_[:], ng[:], scc[:], 0.0, OP.mult, OP.add)

    # x = mp + (u - cp) * ap   -> write into merge2 buffer Q half
    xt = wk_pool.tile([128, W * 64], F32, tag="xt")
    nc.vector.tensor_tensor(xt[:], us[:], cp_[:], OP.subtract)
    xta = wk_pool.tile([128, W * 64], F32, tag="xta")
    nc.vector.tensor_tensor(xta[:], xt[:], ap_[:], OP.mult)
    m2a = wk_pool.tile([128, W * 128], F32, tag="m2a")
    m2b = wk_pool.tile([128, W * 128], F32, tag="m2b")
    m2a_b = m2a[:].rearrange("p (w c) -> p w c", w=W)
    nc.vector.tensor_tensor(m2a_b[:, :, :64],
                            xta[:].rearrange("p (w c) -> p w c", w=W),
                            mp_[:].rearrange("p (w c) -> p w c", w=W), OP.add)
    # upper half: s_vals reversed (descending)
    nc.scalar.copy(m2a_b[:, :, 64:], sv_b[:, :, ::-1])

    bufs2 = [m2a, m2b]
    cur = 0
    for si, d in enumerate([64, 32, 16, 8, 4, 2, 1]):
        src = bufs2[cur][:].rearrange("p (w b two d) -> p w b two d",
                                      w=W, two=2, d=d)
        dst = bufs2[1 - cur][:].rearrange("p (w b two d) -> p w b two d",
                                          w=W, two=2, d=d)
        lo, hi = src[:, :, :, 0, :], src[:, :, :, 1, :]
        eng_mn, eng_mx = merge_split[si]
        eng_mn.tensor_tensor(dst[:, :, :, 0, :], lo, hi, OP.min)
        eng_mx.tensor_tensor(dst[:, :, :, 1, :], lo, hi, OP.max)
        cur = 1 - cur
    zf = bufs2[cur]

    # --- outputs ----------------------------------------------------------
    zslice = dram["z_out"][r0:r0 + ST_RAYS, :].rearrange(
        "(p w) c -> p (w c)", w=W)
    sslice = dram["s_out"][r0:r0 + ST_RAYS, :].rearrange(
        "(p w) c -> p (w c)", w=W)
    nc.sync.dma_start(zslice, zf[:])
    nc.sync.dma_start(sslice, zf[:])

    pts = wk_pool.tile([128, W * 384], F32, tag="pts")
    zf_b = zf[:].rearrange("p (w c) -> p w c", w=W)
    pts_w = pts[:].rearrange("p (w k c) -> p w k c", w=W, c=3)
    for w in range(W):
        for c in range(3):
            nc.scalar.activation(pts_w[:, w, :, c], zf_b[:, w, :],
                                 ACTF.Identity,
                                 bias=od[:, w * 6 + c:w * 6 + c + 1],
                                 scale=od[:, w * 6 + 3 + c:w * 6 + 4 + c])
    ptsl = dram["pts_out"][r0:r0 + ST_RAYS, :].rearrange(
        "(p w) c -> p (w c)", w=W)
    nc.sync.dma_start(ptsl, pts[:])


def build_program(n_rays):
    """Build + compile the per-core program for n_rays rays."""
    assert n_rays % ST_RAYS == 0
    nst = n_rays // ST_RAYS
    nc = bacc.Bacc("TRN2", target_bir_lowering=False, debug=False,
                   enable_asserts=False)
    dram = {}
    for name, width in (("s_vals", 64), ("weights", 64), ("u_in", 64),
                        ("od_in", 6)):
        dram[name] = nc.dram_tensor(name, [n_rays, width], F32,
                                    kind="ExternalInput").ap()
    dram["_const"] = nc.dram_tensor("consts", [128, _C_TOTAL], F32,
                                    kind="ExternalInput").ap()
    for name, width in (("z_out", 128), ("s_out", 128), ("pts_out", 384)):
        dram[name] = nc.dram_tensor(name, [n_rays, width], F32,
                                    kind="ExternalOutput").ap()

    with tile.TileContext(nc) as tc:
        with (tc.tile_pool(name="io", bufs=4) as io_pool,
              tc.tile_pool(name="wk", bufs=2) as wk_pool,
              tc.tile_pool(name="cn", bufs=1) as cn_pool):
            cb = cn_pool.tile([128, _C_TOTAL], F32, tag="cb")
            nc.sync.dma_start(cb[:], dram["_const"])
            dram["_const_sb"] = cb[:]
            neg1 = cn_pool.tile([128, 1], F32, tag="neg1")
            nc.vector.memset(neg1[:], -1.0)
            dram["_neg1"] = neg1[:]
            # merge stage engine split: (min_engine, max_engine) per stage
            v = nc.vector
            merge_split = [(v, v)] * 7
            for st in range(nst):
                _supertile(tc, pools=(io_pool, wk_pool), dram=dram, st=st,
                           merge_split=merge_split)
    nc.compile()
    return nc


@functools.lru_cache(maxsize=2)
def _compiled(n_rays):
    return build_program(n_rays)


# ---------------------------------------------------------------- entry point


def _run(inputs, n_cores=N_CORES, trace=False, trace_kwargs=None):
    rays_o = np.ascontiguousarray(inputs["rays_o"], dtype=np.float32)
    rays_d = np.ascontiguousarray(inputs["rays_d"], dtype=np.float32)
    s_vals = np.ascontiguousarray(inputs["s_vals"], dtype=np.float32)
    weights = np.ascontiguousarray(inputs["weights"], dtype=np.float32)
    b = s_vals.shape[0]
    bpc = b // n_cores
    od = np.concatenate([rays_o, rays_d], axis=1)
    u_t = _u_sorted_tagged()[:b]
    consts = _build_consts()

    nc = _compiled(bpc)
    in_maps = []
    for c in range(n_cores):
        sl = slice(c * bpc, (c + 1) * bpc)
        in_maps.append({
            "s_vals": s_vals[sl],
            "weights": weights[sl],
            "u_in": u_t[sl],
            "od_in": od[sl],
            "consts": consts,
        })
    res = run_bass_kernel_spmd(nc, in_maps, list(range(n_cores)),
                               trace=trace, **(trace_kwargs or {}))
    z = np.concatenate([r["z_out"] for r in res.results], axis=0)
    s = np.concatenate([r["s_out"] for r in res.results], axis=0)
    pts = np.concatenate([r["pts_out"] for r in res.results],
                         axis=0).reshape(b, NS_OUT, 3)
    return (pts, z, s), res


def kernel(rays_o, rays_d, s_vals, weights):
    (pts, z, s), _ = _run({"rays_o": rays_o, "rays_d": rays_d,
                           "s_vals": s_vals, "weights": weights})
    return pts, z, s


# revision 10
# speedup vs baseline: 1.6777x; 1.0007x over previous
"""Trainium2 Bass kernel for nn_NewSampler: PDF importance sampling + merge-sort.

Contract: kernel(**inputs) takes FULL inputs (rays_o, rays_d, s_vals, weights),
returns (pts, z, s) matching reference.py. Shards rays across 8 NeuronCores.

Algorithm (per ray, all on device):
  cdf = cumsum((weights[1:-1]+TINY)/sum)                    [scan]
  u_sorted (host constant, key 42)  ->  s_fine = invCDF(u)  [tagged bitonic
    merge of u with cdf + gated scans + GPSIMD local_scatter seed placement]
  z = merge(s_fine, s_vals)                                 [bitonic merge]
  pts = o + d*z                                             [ACT scale/bias]

u from jax.random.key(42) is input-independent -> host presorts it; invCDF is
monotone, so s_fine comes out sorted and the final sort is a 7-stage merge.
"""

import os
import sys
import functools

import numpy as np

for _p in ("/opt/trn_rl_repo", "/opt/pypackages"):
    if _p not in sys.path:
        sys.path.append(_p)

import concourse.bacc as bacc
import concourse.bass as bass
import concourse.tile as tile
import concourse.mybir as mybir
from concourse._compat import with_exitstack
from concourse.bass_utils import run_bass_kernel_spmd

F32 = mybir.dt.float32
I32 = mybir.dt.int32
I16 = mybir.dt.int16
OP = mybir.AluOpType
ACTF = mybir.ActivationFunctionType

TINY = 1e-6
B_FULL = 131072
NC_SAMP = 64          # coarse samples per ray
NS_OUT = 128          # output samples per ray
N_CORES = 8
BPC = B_FULL // N_CORES   # rays per core
W = 4                 # rays packed per partition
ST_RAYS = 128 * W     # rays per supertile = 512
INF = float("inf")

# ---------------------------------------------------------------- host consts


@functools.lru_cache(maxsize=1)
def _u_sorted_tagged():
    """Sorted per-ray uniforms from key 42 (input-independent), LSB set to 1
    as the merge tag (u entries tag=1, cdf entries tag=0)."""
    import jax
    import jax.numpy as jnp

    cpu = jax.devices("cpu")[0]
    with jax.default_device(cpu):
        u = jax.random.uniform(jax.random.key(42), (B_FULL, NC_SAMP),
                               dtype=jnp.float32)
        u = np.asarray(jax.device_get(u))
    us = np.sort(u, axis=-1)
    ut = (us.view(np.uint32) | np.uint32(1)).view(np.float32)
    return np.ascontiguousarray(ut)


def _build_consts():
    """(128, CF) fp32 const block, identical across partitions."""
    cols = []
    # RG248: cdf-scan reset gate, per 62-block [0, 1*61] x W
    rg248 = np.tile(np.r_[0.0, np.ones(61)], W)
    cols.append(rg248)
    # RG512: merged-scan reset gate per 128-block
    rg512 = np.tile(np.r_[0.0, np.ones(127)], W)
    cols.append(rg512)
    # POS512: q_local + 1 per 128-block
    pos512 = np.tile(np.arange(1, 129, dtype=np.float64), W)
    cols.append(pos512)
    # WB512: per 128-block constant (w*64 - 1)  [for idxB]
    wb512 = np.repeat(np.arange(W) * 64.0 - 1.0, 128)
    cols.append(wb512)
    # IDX496: per (w, k, h): 2*(w*64) + h + 1   [for idxC, +1 folds mask]
    idx496 = np.zeros(W * 62 * 2)
    for w in range(W):
        for k in range(62):
            for h in range(2):
                idx496[(w * 62 + k) * 2 + h] = 2.0 * (w * 64) + h + 1.0
    cols.append(idx496)
    cf = np.concatenate(cols).astype(np.float32)
    return np.ascontiguousarray(np.broadcast_to(cf, (128, cf.size)).copy())


_C_RG248 = 0
_C_RG512 = _C_RG248 + 248
_C_POS512 = _C_RG512 + 512
_C_WB512 = _C_POS512 + 512
_C_IDX496 = _C_WB512 + 512
_C_TOTAL = _C_IDX496 + 496


# ---------------------------------------------------------------- the program


def _block(ap, w, width, n=1):
    """cols [w*width, w*width + n*width) of a per-block-major tile view."""
    return ap[:, w * width:(w + n) * width]


@with_exitstack
def _supertile(ctx, tc, pools, dram, st, merge_split):
    """Emit one supertile (512 rays). dram: dict of DRAM APs."""
    nc = tc.nc
    io_pool, wk_pool = pools
    r0 = st * ST_RAYS

    def dslice(name, width):
        return dram[name][r0:r0 + ST_RAYS, :].rearrange(
            "(p w) c -> p (w c)", w=W)

    sv = io_pool.tile([128, W * 64], F32, tag="sv")
    wt = io_pool.tile([128, W * 64], F32, tag="wt")
    us = io_pool.tile([128, W * 64], F32, tag="us")
    od = io_pool.tile([128, W * 6], F32, tag="od")
    nc.sync.dma_start(sv[:], dslice("s_vals", 64))
    nc.sync.dma_start(wt[:], dslice("weights", 64))
    nc.sync.dma_start(us[:], dslice("u_in", 64))
    nc.sync.dma_start(od[:], dslice("od_in", 6))

    CONST = dram["_const_sb"]

    def cview(off, n):
        return CONST[:, off:off + n]

    # --- tables -----------------------------------------------------------
    # views of the W*64 layout picking 62 interior weights per block
    wt_b = wt[:].rearrange("p (w c) -> p w c", w=W)
    sv_b = sv[:].rearrange("p (w c) -> p w c", w=W)

    wp = wk_pool.tile([128, W * 62], F32, tag="wp")
    wp_b = wp[:].rearrange("p (w c) -> p w c", w=W)
    nc.vector.tensor_scalar(wp_b, wt_b[:, :, 1:63], TINY, None, OP.add)

    tsum = wk_pool.tile([128, W], F32, tag="tsum")
    nc.vector.tensor_reduce(tsum[:].rearrange("p (w c) -> p w c", c=1),
                            wp_b, mybir.AxisListType.X, OP.add)
    rts = wk_pool.tile([128, W], F32, tag="rts")
    nc.vector.reciprocal(rts[:], tsum[:])

    pdf = wk_pool.tile([128, W * 62], F32, tag="pdf")
    # broadcast rT along the 62-dim: iterate (k outer, w inner) so that the
    # broadcast operand keeps innermost stride 1
    pdf_kw = pdf[:].rearrange("p (w c) -> p c w", w=W)
    wp_kw = wp[:].rearrange("p (w c) -> p c w", w=W)
    rts_kw = rts[:].unsqueeze(1).broadcast_to([128, 62, W])
    nc.vector.tensor_tensor(pdf_kw, wp_kw, rts_kw, OP.mult)

    cdf = wk_pool.tile([128, W * 62], F32, tag="cdf")
    nc.vector.tensor_tensor_scan(cdf[:], cview(_C_RG248, 248), pdf[:], 0.0,
                                 OP.mult, OP.add)
    cdf_b = cdf[:].rearrange("p (w c) -> p w c", w=W)

    # mid63: 0.5*(sv[k]+sv[k+1]), k=0..62 per block
    svh = wk_pool.tile([128, W * 64], F32, tag="svh")
    nc.scalar.activation(svh[:], sv[:], ACTF.Copy, scale=0.5)
    svh_b = svh[:].rearrange("p (w c) -> p w c", w=W)
    mid63 = wk_pool.tile([128, W * 63], F32, tag="mid63")
    mid63_b = mid63[:].rearrange("p (w c) -> p w c", w=W)
    nc.vector.tensor_tensor(mid63_b, svh_b[:, :, :63], svh_b[:, :, 1:],
                            OP.add)

    # mid-seed (contiguous k=0..61) for scatter
    midseed = wk_pool.tile([128, W * 62], F32, tag="midseed")
    midseed_b = midseed[:].rearrange("p (w c) -> p w c", w=W)
    nc.scalar.copy(midseed_b, mid63_b[:, :, :62])

    # dmid_k = mid[k+1] - mid[k] + TINY
    dmid = wk_pool.tile([128, W * 62], F32, tag="dmid")
    dmid_b = dmid[:].rearrange("p (w c) -> p w c", w=W)
    nc.vector.scalar_tensor_tensor(dmid_b, mid63_b[:, :, 1:], TINY,
                                   mid63_b[:, :, :62], OP.add, OP.subtract)

    # cklo = [0, cdf_1..cdf_61] per block (contiguous, doubles as C-seed)
    cklo = wk_pool.tile([128, W * 62], F32, tag="cklo")
    cklo_b = cklo[:].rearrange("p (w c) -> p w c", w=W)
    nc.vector.memset(cklo_b[:, :, 0:1], 0.0)
    nc.scalar.copy(cklo_b[:, :, 1:], cdf_b[:, :, :61])

    # D_k = cdf_{k+1} - C_k ; Deff = denom<TINY ? 1 : denom
    dd = wk_pool.tile([128, W * 62], F32, tag="dd")
    nc.vector.tensor_tensor(dd[:].rearrange("p (w c) -> p w c", w=W),
                            cdf_b, cklo_b, OP.subtract)
    deg = wk_pool.tile([128, W * 62], F32, tag="deg")
    nc.vector.tensor_scalar(deg[:], dd[:], TINY, None, OP.is_lt)
    onemd = wk_pool.tile([128, W * 62], F32, tag="onemd")
    nc.vector.tensor_scalar(onemd[:], dd[:], -1.0, 1.0, OP.mult, OP.add)
    degd = wk_pool.tile([128, W * 62], F32, tag="degd")
    nc.vector.tensor_tensor(degd[:], deg[:], onemd[:], OP.mult)
    deff = wk_pool.tile([128, W * 62], F32, tag="deff")
    nc.vector.tensor_tensor(deff[:], dd[:], degd[:], OP.add)

    rdd = wk_pool.tile([128, W * 62], F32, tag="rdd")
    rscr = wk_pool.tile([128, W * 62], F32, tag="rscr")
    nc.vector.reciprocal_approx_accurate(rdd[:], deff[:], rscr[:])

    aseed = wk_pool.tile([128, W * 62], F32, tag="aseed")
    nc.vector.tensor_tensor(aseed[:], dmid[:], rdd[:], OP.mult)

    # --- merge #1: u (tag LSB=1) vs cdf_1..61 (tag LSB=0) -----------------
    m1a = wk_pool.tile([128, W * 128], F32, tag="m1a")
    m1b = wk_pool.tile([128, W * 128], F32, tag="m1b")
    m1a_b = m1a[:].rearrange("p (w c) -> p w c", w=W)
    # Q half: u already tagged host-side
    nc.scalar.copy(m1a_b[:, :, 0:64], us[:].rearrange("p (w c) -> p w c", w=W))
    # C half: [inf, inf, inf, cdf_61..cdf_1] ; clear LSB as tag 0
    nc.vector.memset(m1a_b[:, :, 64:67], INF)
    crev = cdf_b[:, :, 60::-1]  # cdf_61 .. cdf_1
    nc.vector.tensor_scalar(
        m1a_b.bitcast(I32)[:, :, 67:], crev.bitcast(I32), -2, None,
        OP.bitwise_and)

    bufs = [m1a, m1b]
    cur = 0
    for si, d in enumerate([64, 32, 16, 8, 4, 2, 1]):
        src = bufs[cur][:].rearrange("p (w b two d) -> p w b two d",
                                     w=W, two=2, d=d)
        dst = bufs[1 - cur][:].rearrange("p (w b two d) -> p w b two d",
                                         w=W, two=2, d=d)
        lo, hi = src[:, :, :, 0, :], src[:, :, :, 1, :]
        eng_mn, eng_mx = merge_split[si]
        eng_mn.tensor_tensor(dst[:, :, :, 0, :], lo, hi, OP.min)
        eng_mx.tensor_tensor(dst[:, :, :, 1, :], lo, hi, OP.max)
        cur = 1 - cur
    mkeys = bufs[cur]  # merged (128, W*128)

    # --- post-merge scans --------------------------------------------------
    tagi = wk_pool.tile([128, W * 128], I32, tag="tagi")
    nc.vector.tensor_scalar(tagi[:], mkeys[:].bitcast(I32), 1, 1,
                            OP.bitwise_and, OP.bitwise_xor)
    tagc = wk_pool.tile([128, W * 128], F32, tag="tagc")   # 1 at cdf slots
    nc.scalar.copy(tagc[:], tagi[:])

    kcount = wk_pool.tile([128, W * 128], F32, tag="kcount")
    nc.vector.tensor_tensor_scan(kcount[:], cview(_C_RG512, 512), tagc[:],
                                 0.0, OP.mult, OP.add)
    icount = wk_pool.tile([128, W * 128], F32, tag="icount")
    nc.vector.tensor_tensor(icount[:], cview(_C_POS512, 512), kcount[:],
                            OP.subtract)

    # idxB = tagc ? (w*64 + kcount - 1) : -1
    t1 = wk_pool.tile([128, W * 128], F32, tag="t1")
    nc.vector.tensor_tensor(t1[:], kcount[:], cview(_C_WB512, 512), OP.add)
    t2 = wk_pool.tile([128, W * 128], F32, tag="t2")
    nc.vector.tensor_tensor(t2[:], tagc[:], t1[:], OP.mult)
    idxbf = wk_pool.tile([128, W * 128], F32, tag="idxbf")
    nc.vector.scalar_tensor_tensor(idxbf[:], tagc[:], -1.0, t2[:],
                                   OP.add, OP.add)
    idxb16 = wk_pool.tile([128, W * 128], I16, tag="idxb16")
    nc.scalar.copy(idxb16[:], idxbf[:])
    datb16 = wk_pool.tile([128, W * 128], I16, tag="datb16")
    nc.scalar.copy(datb16[:], icount[:])

    f16 = wk_pool.tile([128, W * 64], I16, tag="f16")
    nc.gpsimd.local_scatter(f16[:], datb16[:], idxb16[:], channels=128,
                            num_elems=W * 64, num_idxs=W * 128)

    # posf: per block 63 cols: [0, F_1..F_61, 64.0]
    posf = wk_pool.tile([128, W * 63], F32, tag="posf")
    posf_b = posf[:].rearrange("p (w c) -> p w c", w=W)
    nc.vector.memset(posf_b[:, :, 0:1], 0.0)
    nc.vector.memset(posf_b[:, :, 62:63], 64.0)
    f16_b = f16[:].rearrange("p (w c) -> p w c", w=W)
    nc.scalar.copy(posf_b[:, :, 1:62], f16_b[:, :, 0:61])

    ne = wk_pool.tile([128, W * 62], F32, tag="ne")
    nc.vector.tensor_tensor(ne[:].rearrange("p (w c) -> p w c", w=W),
                            posf_b[:, :, :62], posf_b[:, :, 1:], OP.is_lt)

    # idxC halves: ne ? 2*(w*64 + pos_k) + h : -1   (shared by 3 scatters)
    pos2 = wk_pool.tile([128, W * 62], F32, tag="pos2")
    nc.vector.tensor_scalar(pos2[:].rearrange("p (w c) -> p w c", w=W),
                            posf_b[:, :, :62], 2.0, None, OP.mult)
    idxcf = wk_pool.tile([128, W * 124], F32, tag="idxcf")
    idxcf_h = idxcf[:].rearrange("p (w c h) -> p (w c) h", h=2, w=W)
    pos2_h = pos2[:].unsqueeze(2).broadcast_to([128, W * 62, 2])
    nc.vector.tensor_tensor(idxcf_h, pos2_h,
                            cview(_C_IDX496, 496).rearrange(
                                "p (c h) -> p c h", h=2), OP.add)
    ne_h = ne[:].unsqueeze(2).broadcast_to([128, W * 62, 2])
    idxcm = wk_pool.tile([128, W * 124], F32, tag="idxcm")
    nc.vector.tensor_tensor(idxcm[:].rearrange("p (c h) -> p c h", h=2),
                            idxcf_h, ne_h, OP.mult)
    idxc16 = wk_pool.tile([128, W * 124], I16, tag="idxc16")
    nc.scalar.activation(idxc16[:], idxcm[:], ACTF.Identity,
                         bias=dram["_neg1"])

    # three seed scatters into i-space (64 fp32 = 128 halves per block)
    sca = wk_pool.tile([128, W * 64], F32, tag="sca")
    scm = wk_pool.tile([128, W * 64], F32, tag="scm")
    scc = wk_pool.tile([128, W * 64], F32, tag="scc")
    for dst_t, src_t in ((sca, aseed), (scm, midseed), (scc, cklo)):
        nc.gpsimd.local_scatter(dst_t[:].bitcast(I16), src_t[:].bitcast(I16),
                                idxc16[:], channels=128, num_elems=W * 128,
                                num_idxs=W * 124)

    # gate / propagate
    gate = wk_pool.tile([128, W * 64], F32, tag="gate")
    nc.vector.tensor_scalar(gate[:], scm[:], 1.0, None, OP.min)
    ng = wk_pool.tile([128, W * 64], F32, tag="ng")
    nc.vector.tensor_scalar(ng[:], gate[:], -1.0, 1.0, OP.mult, OP.add)
    ap_ = wk_pool.tile([128, W * 64], F32, tag="ap_")
    mp_ = wk_pool.tile([128, W * 64], F32, tag="mp_")
    cp_ = wk_pool.tile([128, W * 64], F32, tag="cp_")
    nc.vector.tensor_tensor_scan(ap_[:], ng[:], sca[:], 0.0, OP.mult, OP.add)
    nc.vector.tensor_tensor_scan(mp_[:], ng[:], scm[:], 0.0, OP.mult, OP.add)
    nc.vector.tensor_tensor_scan(cp_[:], ng[:], scc[:], 0.0, OP.mult, OP.add)

    # x = mp + (u - cp) * ap   -> write into merge2 buffer Q half
    xt = wk_pool.tile([128, W * 64], F32, tag="xt")
    nc.vector.tensor_tensor(xt[:], us[:], cp_[:], OP.subtract)
    xta = wk_pool.tile([128, W * 64], F32, tag="xta")
    nc.vector.tensor_tensor(xta[:], xt[:], ap_[:], OP.mult)
    m2a = wk_pool.tile([128, W * 128], F32, tag="m2a")
    m2b = wk_pool.tile([128, W * 128], F32, tag="m2b")
    m2a_b = m2a[:].rearrange("p (w c) -> p w c", w=W)
    nc.vector.tensor_tensor(m2a_b[:, :, :64],
                            xta[:].rearrange("p (w c) -> p w c", w=W),
                            mp_[:].rearrange("p (w c) -> p w c", w=W), OP.add)
    # upper half: s_vals reversed (descending)
    nc.scalar.copy(m2a_b[:, :, 64:], sv_b[:, :, ::-1])

    bufs2 = [m2a, m2b]
    cur = 0
    for si, d in enumerate([64, 32, 16, 8, 4, 2, 1]):
        src = bufs2[cur][:].rearrange("p (w b two d) -> p w b two d",
                                      w=W, two=2, d=d)
        dst = bufs2[1 - cur][:].rearrange("p (w b two d) -> p w b two d",
                                          w=W, two=2, d=d)
        lo, hi = src[:, :, :, 0, :], src[:, :, :, 1, :]
        eng_mn, eng_mx = merge_split[si]
        eng_mn.tensor_tensor(dst[:, :, :, 0, :], lo, hi, OP.min)
        eng_mx.tensor_tensor(dst[:, :, :, 1, :], lo, hi, OP.max)
        cur = 1 - cur
    zf = bufs2[cur]

    # --- outputs ----------------------------------------------------------
    zslice = dram["z_out"][r0:r0 + ST_RAYS, :].rearrange(
        "(p w) c -> p (w c)", w=W)
    sslice = dram["s_out"][r0:r0 + ST_RAYS, :].rearrange(
        "(p w) c -> p (w c)", w=W)
    nc.sync.dma_start(zslice, zf[:])
    nc.sync.dma_start(sslice, zf[:])

    pts = wk_pool.tile([128, W * 384], F32, tag="pts")
    zf_b = zf[:].rearrange("p (w c) -> p w c", w=W)
    pts_w = pts[:].rearrange("p (w k c) -> p w k c", w=W, c=3)
    for w in range(W):
        for c in range(3):
            nc.scalar.activation(pts_w[:, w, :, c], zf_b[:, w, :],
                                 ACTF.Identity,
                                 bias=od[:, w * 6 + c:w * 6 + c + 1],
                                 scale=od[:, w * 6 + 3 + c:w * 6 + 4 + c])
    ptsl = dram["pts_out"][r0:r0 + ST_RAYS, :].rearrange(
        "(p w) c -> p (w c)", w=W)
    nc.sync.dma_start(ptsl, pts[:])


def build_program(n_rays):
    """Build + compile the per-core program for n_rays rays."""
    assert n_rays % ST_RAYS == 0
    nst = n_rays // ST_RAYS
    nc = bacc.Bacc("TRN2", target_bir_lowering=False, debug=False,
                   enable_asserts=False)
    dram = {}
    for name, width in (("s_vals", 64), ("weights", 64), ("u_in", 64),
                        ("od_in", 6)):
        dram[name] = nc.dram_tensor(name, [n_rays, width], F32,
                                    kind="ExternalInput").ap()
    dram["_const"] = nc.dram_tensor("consts", [128, _C_TOTAL], F32,
                                    kind="ExternalInput").ap()
    for name, width in (("z_out", 128), ("s_out", 128), ("pts_out", 384)):
        dram[name] = nc.dram_tensor(name, [n_rays, width], F32,
                                    kind="ExternalOutput").ap()

    with tile.TileContext(nc) as tc:
        with (tc.tile_pool(name="io", bufs=4) as io_pool,
              tc.tile_pool(name="wk", bufs=2) as wk_pool,
              tc.tile_pool(name="cn", bufs=1) as cn_pool):
            cb = cn_pool.tile([128, _C_TOTAL], F32, tag="cb")
            nc.sync.dma_start(cb[:], dram["_const"])
            dram["_const_sb"] = cb[:]
            neg1 = cn_pool.tile([128, 1], F32, tag="neg1")
            nc.vector.memset(neg1[:], -1.0)
            dram["_neg1"] = neg1[:]
            # merge stage engine split: (min_engine, max_engine) per stage
            v = nc.vector
            merge_split = [(v, v)] * 7
            for st in range(nst):
                _supertile(tc, pools=(io_pool, wk_pool), dram=dram, st=st,
                           merge_split=merge_split)
    nc.compile()
    return nc


@functools.lru_cache(maxsize=2)
def _compiled(n_rays):
    return build_program(n_rays)


# ---------------------------------------------------------------- entry point


def _run(inputs, n_cores=N_CORES, trace=False, trace_kwargs=None):
    rays_o = np.ascontiguousarray(inputs["rays_o"], dtype=np.float32)
    rays_d = np.ascontiguousarray(inputs["rays_d"], dtype=np.float32)
    s_vals = np.ascontiguousarray(inputs["s_vals"], dtype=np.float32)
    weights = np.ascontiguousarray(inputs["weights"], dtype=np.float32)
    b = s_vals.shape[0]
    bpc = b // n_cores
    od = np.concatenate([rays_o, rays_d], axis=1)
    u_t = _u_sorted_tagged()[:b]
    consts = _build_consts()

    nc = _compiled(bpc)
    in_maps = []
    for c in range(n_cores):
        sl = slice(c * bpc, (c + 1) * bpc)
        in_maps.append({
            "s_vals": s_vals[sl],
            "weights": weights[sl],
            "u_in": u_t[sl],
            "od_in": od[sl],
            "consts": consts,
        })
    res = run_bass_kernel_spmd(nc, in_maps, list(range(n_cores)),
                               trace=trace, **(trace_kwargs or {}))
    z = np.concatenate([r["z_out"] for r in res.results], axis=0)
    s = np.concatenate([r["s_out"] for r in res.results], axis=0)
    pts = np.concatenate([r["pts_out"] for r in res.results],
                         axis=0).reshape(b, NS_OUT, 3)
    return (pts, z, s), res


def kernel(rays_o, rays_d, s_vals, weights):
    (pts, z, s), _ = _run({"rays_o": rays_o, "rays_d": rays_d,
                           "s_vals": s_vals, "weights": weights})
    return pts, z, s


# revision 11
# speedup vs baseline: 1.7270x; 1.0293x over previous
"""Trainium2 Bass kernel for nn_NewSampler: PDF importance sampling + merge-sort.

Contract: kernel(**inputs) takes FULL inputs (rays_o, rays_d, s_vals, weights),
returns (pts, z, s) matching reference.py. Shards rays across 8 NeuronCores.

Algorithm (per ray, all on device):
  cdf = cumsum((weights[1:-1]+TINY)/sum)                    [scan]
  u_sorted (host constant, key 42)  ->  s_fine = invCDF(u)  [tagged bitonic
    merge of u with cdf + gated scans + GPSIMD local_scatter seed placement]
  z = merge(s_fine, s_vals)                                 [bitonic merge]
  pts = o + d*z                                             [ACT scale/bias]

u from jax.random.key(42) is input-independent -> host presorts it; invCDF is
monotone, so s_fine comes out sorted and the final sort is a 7-stage merge.
"""

import os
import sys
import functools

import numpy as np

for _p in ("/opt/trn_rl_repo", "/opt/pypackages"):
    if _p not in sys.path:
        sys.path.append(_p)

import concourse.bacc as bacc
import concourse.bass as bass
import concourse.tile as tile
import concourse.mybir as mybir
from concourse._compat import with_exitstack
from concourse.bass_utils import run_bass_kernel_spmd

F32 = mybir.dt.float32
I32 = mybir.dt.int32
I16 = mybir.dt.int16
OP = mybir.AluOpType
ACTF = mybir.ActivationFunctionType

TINY = 1e-6
B_FULL = 131072
NC_SAMP = 64          # coarse samples per ray
NS_OUT = 128          # output samples per ray
N_CORES = 8
BPC = B_FULL // N_CORES   # rays per core
W = 4                 # rays packed per partition
ST_RAYS = 128 * W     # rays per supertile = 512
INF = float("inf")

# ---------------------------------------------------------------- host consts


@functools.lru_cache(maxsize=1)
def _u_sorted_tagged():
    """Sorted per-ray uniforms from key 42 (input-independent), LSB set to 1
    as the merge tag (u entries tag=1, cdf entries tag=0)."""
    import jax
    import jax.numpy as jnp

    cpu = jax.devices("cpu")[0]
    with jax.default_device(cpu):
        u = jax.random.uniform(jax.random.key(42), (B_FULL, NC_SAMP),
                               dtype=jnp.float32)
        u = np.asarray(jax.device_get(u))
    us = np.sort(u, axis=-1)
    ut = (us.view(np.uint32) | np.uint32(1)).view(np.float32)
    return np.ascontiguousarray(ut)


def _build_consts():
    """(128, CF) fp32 const block, identical across partitions."""
    cols = []
    # RG248: cdf-scan reset gate, per 62-block [0, 1*61] x W
    rg248 = np.tile(np.r_[0.0, np.ones(61)], W)
    cols.append(rg248)
    # RG512: merged-scan reset gate per 128-block
    rg512 = np.tile(np.r_[0.0, np.ones(127)], W)
    cols.append(rg512)
    # POS512: q_local + 1 per 128-block
    pos512 = np.tile(np.arange(1, 129, dtype=np.float64), W)
    cols.append(pos512)
    # WB512: per 128-block constant (w*64)  [idxB = tagc*(kcount+w*64) - 1]
    wb512 = np.repeat(np.arange(W) * 64.0, 128)
    cols.append(wb512)
    # IDX496: per (w, k, h): 2*(w*64) + h + 1   [for idxC, +1 folds mask]
    idx496 = np.zeros(W * 62 * 2)
    for w in range(W):
        for k in range(62):
            for h in range(2):
                idx496[(w * 62 + k) * 2 + h] = 2.0 * (w * 64) + h + 1.0
    cols.append(idx496)
    cf = np.concatenate(cols).astype(np.float32)
    return np.ascontiguousarray(np.broadcast_to(cf, (128, cf.size)).copy())


_C_RG248 = 0
_C_RG512 = _C_RG248 + 248
_C_POS512 = _C_RG512 + 512
_C_WB512 = _C_POS512 + 512
_C_IDX496 = _C_WB512 + 512
_C_TOTAL = _C_IDX496 + 496


# ---------------------------------------------------------------- the program


def _block(ap, w, width, n=1):
    """cols [w*width, w*width + n*width) of a per-block-major tile view."""
    return ap[:, w * width:(w + n) * width]


@with_exitstack
def _supertile(ctx, tc, pools, dram, st, merge_split):
    """Emit one supertile (512 rays). dram: dict of DRAM APs."""
    nc = tc.nc
    io_pool, wk_pool = pools
    r0 = st * ST_RAYS

    def dslice(name, width):
        return dram[name][r0:r0 + ST_RAYS, :].rearrange(
            "(p w) c -> p (w c)", w=W)

    sv = io_pool.tile([128, W * 64], F32, tag="sv")
    wt = io_pool.tile([128, W * 64], F32, tag="wt")
    us = io_pool.tile([128, W * 64], F32, tag="us")
    od = io_pool.tile([128, W * 6], F32, tag="od")
    nc.sync.dma_start(sv[:], dslice("s_vals", 64))
    nc.sync.dma_start(wt[:], dslice("weights", 64))
    nc.sync.dma_start(us[:], dslice("u_in", 64))
    nc.sync.dma_start(od[:], dslice("od_in", 6))

    CONST = dram["_const_sb"]

    def cview(off, n):
        return CONST[:, off:off + n]

    # --- tables -----------------------------------------------------------
    # views of the W*64 layout picking 62 interior weights per block
    wt_b = wt[:].rearrange("p (w c) -> p w c", w=W)
    sv_b = sv[:].rearrange("p (w c) -> p w c", w=W)

    wp = wk_pool.tile([128, W * 62], F32, tag="wp")
    wp_b = wp[:].rearrange("p (w c) -> p w c", w=W)
    nc.vector.tensor_scalar(wp_b, wt_b[:, :, 1:63], TINY, None, OP.add)

    tsum = wk_pool.tile([128, W], F32, tag="tsum")
    nc.vector.tensor_reduce(tsum[:].rearrange("p (w c) -> p w c", c=1),
                            wp_b, mybir.AxisListType.X, OP.add)
    rts = wk_pool.tile([128, W], F32, tag="rts")
    nc.vector.reciprocal(rts[:], tsum[:])

    pdf = wk_pool.tile([128, W * 62], F32, tag="pdf")
    # broadcast rT along the 62-dim: iterate (k outer, w inner) so that the
    # broadcast operand keeps innermost stride 1
    pdf_kw = pdf[:].rearrange("p (w c) -> p c w", w=W)
    wp_kw = wp[:].rearrange("p (w c) -> p c w", w=W)
    rts_kw = rts[:].unsqueeze(1).broadcast_to([128, 62, W])
    nc.vector.tensor_tensor(pdf_kw, wp_kw, rts_kw, OP.mult)

    cdf = wk_pool.tile([128, W * 62], F32, tag="cdf")
    nc.vector.tensor_tensor_scan(cdf[:], cview(_C_RG248, 248), pdf[:], 0.0,
                                 OP.mult, OP.add)
    cdf_b = cdf[:].rearrange("p (w c) -> p w c", w=W)

    # mid63: 0.5*(sv[k]+sv[k+1]), k=0..62 per block
    svh = wk_pool.tile([128, W * 64], F32, tag="svh")
    nc.scalar.activation(svh[:], sv[:], ACTF.Copy, scale=0.5)
    svh_b = svh[:].rearrange("p (w c) -> p w c", w=W)
    mid63 = wk_pool.tile([128, W * 63], F32, tag="mid63")
    mid63_b = mid63[:].rearrange("p (w c) -> p w c", w=W)
    nc.vector.tensor_tensor(mid63_b, svh_b[:, :, :63], svh_b[:, :, 1:],
                            OP.add)

    # mid-seed (contiguous k=0..61) for scatter
    midseed = wk_pool.tile([128, W * 62], F32, tag="midseed")
    midseed_b = midseed[:].rearrange("p (w c) -> p w c", w=W)
    nc.scalar.copy(midseed_b, mid63_b[:, :, :62])

    # dmid_k = mid[k+1] - mid[k] + TINY
    dmid = wk_pool.tile([128, W * 62], F32, tag="dmid")
    dmid_b = dmid[:].rearrange("p (w c) -> p w c", w=W)
    nc.vector.scalar_tensor_tensor(dmid_b, mid63_b[:, :, 1:], TINY,
                                   mid63_b[:, :, :62], OP.add, OP.subtract)

    # cklo = [0, cdf_1..cdf_61] per block (contiguous, doubles as C-seed)
    cklo = wk_pool.tile([128, W * 62], F32, tag="cklo")
    cklo_b = cklo[:].rearrange("p (w c) -> p w c", w=W)
    nc.vector.memset(cklo_b[:, :, 0:1], 0.0)
    nc.scalar.copy(cklo_b[:, :, 1:], cdf_b[:, :, :61])

    # D_k = cdf_{k+1} - C_k ; Deff = denom<TINY ? 1 : denom
    dd = wk_pool.tile([128, W * 62], F32, tag="dd")
    nc.vector.tensor_tensor(dd[:].rearrange("p (w c) -> p w c", w=W),
                            cdf_b, cklo_b, OP.subtract)
    deg = wk_pool.tile([128, W * 62], F32, tag="deg")
    nc.vector.tensor_scalar(deg[:], dd[:], TINY, None, OP.is_lt)
    onemd = wk_pool.tile([128, W * 62], F32, tag="onemd")
    nc.vector.tensor_scalar(onemd[:], dd[:], -1.0, 1.0, OP.mult, OP.add)
    degd = wk_pool.tile([128, W * 62], F32, tag="degd")
    nc.vector.tensor_tensor(degd[:], deg[:], onemd[:], OP.mult)
    deff = wk_pool.tile([128, W * 62], F32, tag="deff")
    nc.vector.tensor_tensor(deff[:], dd[:], degd[:], OP.add)

    rdd = wk_pool.tile([128, W * 62], F32, tag="rdd")
    rscr = wk_pool.tile([128, W * 62], F32, tag="rscr")
    nc.vector.reciprocal_approx_accurate(rdd[:], deff[:], rscr[:])

    aseed = wk_pool.tile([128, W * 62], F32, tag="aseed")
    nc.vector.tensor_tensor(aseed[:], dmid[:], rdd[:], OP.mult)

    # --- merge #1: u (tag LSB=1) vs cdf_1..61 (tag LSB=0) -----------------
    m1a = wk_pool.tile([128, W * 128], F32, tag="m1a")
    m1b = wk_pool.tile([128, W * 128], F32, tag="m1b")
    m1a_b = m1a[:].rearrange("p (w c) -> p w c", w=W)
    # Q half: u already tagged host-side
    nc.scalar.copy(m1a_b[:, :, 0:64], us[:].rearrange("p (w c) -> p w c", w=W))
    # C half: [inf, inf, inf, cdf_61..cdf_1] ; clear LSB as tag 0
    nc.vector.memset(m1a_b[:, :, 64:67], INF)
    crev = cdf_b[:, :, 60::-1]  # cdf_61 .. cdf_1
    nc.vector.tensor_scalar(
        m1a_b.bitcast(I32)[:, :, 67:], crev.bitcast(I32), -2, None,
        OP.bitwise_and)

    bufs = [m1a, m1b]
    cur = 0
    for si, d in enumerate([64, 32, 16, 8, 4, 2, 1]):
        src = bufs[cur][:].rearrange("p (w b two d) -> p w b two d",
                                     w=W, two=2, d=d)
        dst = bufs[1 - cur][:].rearrange("p (w b two d) -> p w b two d",
                                         w=W, two=2, d=d)
        lo, hi = src[:, :, :, 0, :], src[:, :, :, 1, :]
        eng_mn, eng_mx = merge_split[si]
        eng_mn.tensor_tensor(dst[:, :, :, 0, :], lo, hi, OP.min)
        eng_mx.tensor_tensor(dst[:, :, :, 1, :], lo, hi, OP.max)
        cur = 1 - cur
    mkeys = bufs[cur]  # merged (128, W*128)

    # --- post-merge scans --------------------------------------------------
    tagi = wk_pool.tile([128, W * 128], I32, tag="tagi")
    nc.vector.tensor_scalar(tagi[:], mkeys[:].bitcast(I32), 1, 1,
                            OP.bitwise_and, OP.bitwise_xor)
    tagc = wk_pool.tile([128, W * 128], F32, tag="tagc")   # 1 at cdf slots
    nc.scalar.copy(tagc[:], tagi[:])

    kcount = wk_pool.tile([128, W * 128], F32, tag="kcount")
    nc.vector.tensor_tensor_scan(kcount[:], cview(_C_RG512, 512), tagc[:],
                                 0.0, OP.mult, OP.add)
    icount = wk_pool.tile([128, W * 128], F32, tag="icount")
    nc.vector.tensor_tensor(icount[:], cview(_C_POS512, 512), kcount[:],
                            OP.subtract)

    # idxB = tagc ? (w*64 + kcount - 1) : -1
    t1 = wk_pool.tile([128, W * 128], F32, tag="t1")
    nc.vector.tensor_tensor(t1[:], kcount[:], cview(_C_WB512, 512), OP.add)
    t2 = wk_pool.tile([128, W * 128], F32, tag="t2")
    nc.vector.tensor_tensor(t2[:], tagc[:], t1[:], OP.mult)
    idxb16 = wk_pool.tile([128, W * 128], I16, tag="idxb16")
    nc.scalar.activation(idxb16[:], t2[:], ACTF.Identity, bias=dram["_neg1"])
    datb16 = wk_pool.tile([128, W * 128], I16, tag="datb16")
    nc.scalar.copy(datb16[:], icount[:])

    f16 = wk_pool.tile([128, W * 64], I16, tag="f16")
    nc.gpsimd.local_scatter(f16[:], datb16[:], idxb16[:], channels=128,
                            num_elems=W * 64, num_idxs=W * 128)

    # posf: per block 63 cols: [0, F_1..F_61, 64.0]
    posf = wk_pool.tile([128, W * 63], F32, tag="posf")
    posf_b = posf[:].rearrange("p (w c) -> p w c", w=W)
    nc.vector.memset(posf_b[:, :, 0:1], 0.0)
    nc.vector.memset(posf_b[:, :, 62:63], 64.0)
    f16_b = f16[:].rearrange("p (w c) -> p w c", w=W)
    nc.scalar.copy(posf_b[:, :, 1:62], f16_b[:, :, 0:61])

    ne = wk_pool.tile([128, W * 62], F32, tag="ne")
    nc.vector.tensor_tensor(ne[:].rearrange("p (w c) -> p w c", w=W),
                            posf_b[:, :, :62], posf_b[:, :, 1:], OP.is_lt)

    # idxC halves: ne ? 2*(w*64 + pos_k) + h : -1   (shared by 3 scatters)
    pos2 = wk_pool.tile([128, W * 62], F32, tag="pos2")
    nc.vector.tensor_scalar(pos2[:].rearrange("p (w c) -> p w c", w=W),
                            posf_b[:, :, :62], 2.0, None, OP.mult)
    idxcf = wk_pool.tile([128, W * 124], F32, tag="idxcf")
    idxcf_h = idxcf[:].rearrange("p (w c h) -> p (w c) h", h=2, w=W)
    pos2_h = pos2[:].unsqueeze(2).broadcast_to([128, W * 62, 2])
    nc.vector.tensor_tensor(idxcf_h, pos2_h,
                            cview(_C_IDX496, 496).rearrange(
                                "p (c h) -> p c h", h=2), OP.add)
    ne_h = ne[:].unsqueeze(2).broadcast_to([128, W * 62, 2])
    idxcm = wk_pool.tile([128, W * 124], F32, tag="idxcm")
    nc.vector.tensor_tensor(idxcm[:].rearrange("p (c h) -> p c h", h=2),
                            idxcf_h, ne_h, OP.mult)
    idxc16 = wk_pool.tile([128, W * 124], I16, tag="idxc16")
    nc.scalar.activation(idxc16[:], idxcm[:], ACTF.Identity,
                         bias=dram["_neg1"])

    # three seed scatters into i-space (64 fp32 = 128 halves per block)
    sca = wk_pool.tile([128, W * 64], F32, tag="sca")
    scm = wk_pool.tile([128, W * 64], F32, tag="scm")
    scc = wk_pool.tile([128, W * 64], F32, tag="scc")
    for dst_t, src_t in ((sca, aseed), (scm, midseed), (scc, cklo)):
        nc.gpsimd.local_scatter(dst_t[:].bitcast(I16), src_t[:].bitcast(I16),
                                idxc16[:], channels=128, num_elems=W * 128,
                                num_idxs=W * 124)

    # gate / propagate
    gate = wk_pool.tile([128, W * 64], F32, tag="gate")
    nc.vector.tensor_scalar(gate[:], scm[:], 1.0, None, OP.min)
    ng = wk_pool.tile([128, W * 64], F32, tag="ng")
    nc.vector.tensor_scalar(ng[:], gate[:], -1.0, 1.0, OP.mult, OP.add)
    ap_ = wk_pool.tile([128, W * 64], F32, tag="ap_")
    mp_ = wk_pool.tile([128, W * 64], F32, tag="mp_")
    cp_ = wk_pool.tile([128, W * 64], F32, tag="cp_")
    nc.vector.tensor_tensor_scan(ap_[:], ng[:], sca[:], 0.0, OP.mult, OP.add)
    nc.vector.tensor_tensor_scan(mp_[:], ng[:], scm[:], 0.0, OP.mult, OP.add)
    nc.vector.tensor_tensor_scan(cp_[:], ng[:], scc[:], 0.0, OP.mult, OP.add)

    # x = mp + (u - cp) * ap   -> write into merge2 buffer Q half
    xt = wk_pool.tile([128, W * 64], F32, tag="xt")
    nc.vector.tensor_tensor(xt[:], us[:], cp_[:], OP.subtract)
    xta = wk_pool.tile([128, W * 64], F32, tag="xta")
    nc.vector.tensor_tensor(xta[:], xt[:], ap_[:], OP.mult)
    m2a = wk_pool.tile([128, W * 128], F32, tag="m2a")
    m2b = wk_pool.tile([128, W * 128], F32, tag="m2b")
    m2a_b = m2a[:].rearrange("p (w c) -> p w c", w=W)
    nc.vector.tensor_tensor(m2a_b[:, :, :64],
                            xta[:].rearrange("p (w c) -> p w c", w=W),
                            mp_[:].rearrange("p (w c) -> p w c", w=W), OP.add)
    # upper half: s_vals reversed (descending)
    nc.scalar.copy(m2a_b[:, :, 64:], sv_b[:, :, ::-1])

    bufs2 = [m2a, m2b]
    cur = 0
    for si, d in enumerate([64, 32, 16, 8, 4, 2, 1]):
        src = bufs2[cur][:].rearrange("p (w b two d) -> p w b two d",
                                      w=W, two=2, d=d)
        dst = bufs2[1 - cur][:].rearrange("p (w b two d) -> p w b two d",
                                          w=W, two=2, d=d)
        lo, hi = src[:, :, :, 0, :], src[:, :, :, 1, :]
        eng_mn, eng_mx = merge_split[si]
        eng_mn.tensor_tensor(dst[:, :, :, 0, :], lo, hi, OP.min)
        eng_mx.tensor_tensor(dst[:, :, :, 1, :], lo, hi, OP.max)
        cur = 1 - cur
    zf = bufs2[cur]

    # --- outputs ----------------------------------------------------------
    zslice = dram["z_out"][r0:r0 + ST_RAYS, :].rearrange(
        "(p w) c -> p (w c)", w=W)
    sslice = dram["s_out"][r0:r0 + ST_RAYS, :].rearrange(
        "(p w) c -> p (w c)", w=W)
    nc.sync.dma_start(zslice, zf[:])
    nc.sync.dma_start(sslice, zf[:])

    pts = wk_pool.tile([128, W * 384], F32, tag="pts")
    zf_b = zf[:].rearrange("p (w c) -> p w c", w=W)
    pts_w = pts[:].rearrange("p (w k c) -> p w k c", w=W, c=3)
    for w in range(W):
        for c in range(3):
            nc.scalar.activation(pts_w[:, w, :, c], zf_b[:, w, :],
                                 ACTF.Identity,
                                 bias=od[:, w * 6 + c:w * 6 + c + 1],
                                 scale=od[:, w * 6 + 3 + c:w * 6 + 4 + c])
    ptsl = dram["pts_out"][r0:r0 + ST_RAYS, :].rearrange(
        "(p w) c -> p (w c)", w=W)
    nc.sync.dma_start(ptsl, pts[:])


def build_program(n_rays):
    """Build + compile the per-core program for n_rays rays."""
    assert n_rays % ST_RAYS == 0
    nst = n_rays // ST_RAYS
    nc = bacc.Bacc("TRN2", target_bir_lowering=False, debug=False,
                   enable_asserts=False)
    dram = {}
    for name, width in (("s_vals", 64), ("weights", 64), ("u_in", 64),
                        ("od_in", 6)):
        dram[name] = nc.dram_tensor(name, [n_rays, width], F32,
                                    kind="ExternalInput").ap()
    dram["_const"] = nc.dram_tensor("consts", [128, _C_TOTAL], F32,
                                    kind="ExternalInput").ap()
    for name, width in (("z_out", 128), ("s_out", 128), ("pts_out", 384)):
        dram[name] = nc.dram_tensor(name, [n_rays, width], F32,
                                    kind="ExternalOutput").ap()

    with tile.TileContext(nc) as tc:
        with (tc.tile_pool(name="io", bufs=4) as io_pool,
              tc.tile_pool(name="wk", bufs=3) as wk_pool,
              tc.tile_pool(name="cn", bufs=1) as cn_pool):
            cb = cn_pool.tile([128, _C_TOTAL], F32, tag="cb")
            nc.sync.dma_start(cb[:], dram["_const"])
            dram["_const_sb"] = cb[:]
            neg1 = cn_pool.tile([128, 1], F32, tag="neg1")
            nc.vector.memset(neg1[:], -1.0)
            dram["_neg1"] = neg1[:]
            # merge stage engine split: (min_engine, max_engine) per stage
            v = nc.vector
            merge_split = [(v, v)] * 7
            for st in range(nst):
                _supertile(tc, pools=(io_pool, wk_pool), dram=dram, st=st,
                           merge_split=merge_split)
    nc.compile()
    return nc


@functools.lru_cache(maxsize=2)
def _compiled(n_rays):
    return build_program(n_rays)


# ---------------------------------------------------------------- entry point


def _run(inputs, n_cores=N_CORES, trace=False, trace_kwargs=None):
    rays_o = np.ascontiguousarray(inputs["rays_o"], dtype=np.float32)
    rays_d = np.ascontiguousarray(inputs["rays_d"], dtype=np.float32)
    s_vals = np.ascontiguousarray(inputs["s_vals"], dtype=np.float32)
    weights = np.ascontiguousarray(inputs["weights"], dtype=np.float32)
    b = s_vals.shape[0]
    bpc = b // n_cores
    od = np.concatenate([rays_o, rays_d], axis=1)
    u_t = _u_sorted_tagged()[:b]
    consts = _build_consts()

    nc = _compiled(bpc)
    in_maps = []
    for c in range(n_cores):
        sl = slice(c * bpc, (c + 1) * bpc)
        in_maps.append({
            "s_vals": s_vals[sl],
            "weights": weights[sl],
            "u_in": u_t[sl],
            "od_in": od[sl],
            "consts": consts,
        })
    res = run_bass_kernel_spmd(nc, in_maps, list(range(n_cores)),
                               trace=trace, **(trace_kwargs or {}))
    z = np.concatenate([r["z_out"] for r in res.results], axis=0)
    s = np.concatenate([r["s_out"] for r in res.results], axis=0)
    pts = np.concatenate([r["pts_out"] for r in res.results],
                         axis=0).reshape(b, NS_OUT, 3)
    return (pts, z, s), res


def kernel(rays_o, rays_d, s_vals, weights):
    (pts, z, s), _ = _run({"rays_o": rays_o, "rays_d": rays_d,
                           "s_vals": s_vals, "weights": weights})
    return pts, z, s
